# revision 1
# baseline (speedup 1.0000x reference)
"""AttentionMIL pooling kernel for 8 Trainium2 NeuronCores.

Math (per slide b): h = tanh(X @ W1^T); s = h @ w2; a = softmax(s);
out = a^T @ X, with X [N=8192, D=1024], W1 [H=256, D], w2 [H].

Strategy:
  - Data-parallel over the slide dim: 16 slides / 8 cores = 2 per core.
  - Host prep: cast X to bf16 in BOTH layouts (natural [N, D] for the
    attention-weighted sum, transposed [D, N] for the score matmul) so the
    PE never needs on-device transposes of X. W1^T and a replicated w2 row
    are also shipped (tiny).
  - Scores, per 512-row tile: h = XT_chunk^T @ W1T on PE (bf16, fp32 PSUM
    accum over D, 4 blocks side by side in one 2-bank PSUM tile), then ONE
    tanh (ACT), ONE mul by w2 (DVE), ONE 3D reduce -> per-block scores,
    ONE exp (ACT) per tile.
  - Softmax without a max pass: s = w2 . tanh(.) is bounded by ||w2||_1
    (~13 for this data), so exp(s) cannot overflow fp32; l = sum(exp(s))
    and acc = sum(exp(s_n) x_n) accumulate in PSUM across the whole slide,
    out = acc / l at the end.
  - The weighted-sum matmuls for tile t are emitted after the score
    matmuls of tile t+1, giving the ACT/DVE score chain a full tile of
    slack so the PE never stalls on it.
"""

import sys

sys.path.insert(0, "/opt/trn_rl_repo")

import numpy as np
import ml_dtypes

import concourse.bacc as bacc
import concourse.tile as tile
from concourse import mybir
from concourse.bass_utils import run_bass_kernel_spmd

BF16 = ml_dtypes.bfloat16
B, N, D, H = 16, 8192, 1024, 256
NCORES = 8
SPC = B // NCORES          # slides per core
NT = 512                   # rows of N per tile
NBLK = NT // 128           # 128-row blocks per tile
TILES = N // NT
KCH = D // 128             # contraction chunks

_NC_CACHE = {}


def _build_nc():
    bf = mybir.dt.bfloat16
    f32 = mybir.dt.float32
    AF = mybir.ActivationFunctionType

    nc = bacc.Bacc("TRN2", num_devices=NCORES)
    # Host-swizzled layouts: each per-tile DMA reads one fully contiguous
    # 1 MiB region into a [128, free] SBUF tile (128 descriptors x 8 KiB).
    #   xt[s, t, p, k*NT + j]   = X[s, t*NT + j, k*128 + p]   (transposed view)
    #   xn[s, t, p, b*D + j]    = X[s, t*NT + b*128 + p, j]   (natural view)
    xt = nc.declare_dram_parameter("xt", [SPC, TILES, 128, KCH * NT], bf, isOutput=False)
    xn = nc.declare_dram_parameter("xn", [SPC, TILES, 128, NBLK * D], bf, isOutput=False)
    w1t = nc.declare_dram_parameter("w1t", [128, KCH * H], bf, isOutput=False)
    w2r = nc.declare_dram_parameter("w2r", [128, NBLK * H], bf, isOutput=False)
    ones = nc.declare_dram_parameter("ones", [128, 1], bf, isOutput=False)
    outp = nc.declare_dram_parameter("out", [SPC, D], f32, isOutput=True)
    outl = nc.declare_dram_parameter("outl", [SPC, NBLK, 1], f32, isOutput=True)

    with tile.TileContext(nc) as tc:
        with tc.tile_pool(name="const", bufs=1) as constp, \
             tc.tile_pool(name="xt", bufs=6) as xtp, \
             tc.tile_pool(name="xn", bufs=6) as xnp, \
             tc.tile_pool(name="tanh", bufs=2) as tp, \
             tc.tile_pool(name="tmp", bufs=2) as tmpp, \
             tc.tile_pool(name="score", bufs=2) as sp, \
             tc.tile_pool(name="expw", bufs=3) as ep, \
             tc.tile_pool(name="outsb", bufs=2) as outsbp, \
             tc.tile_pool(name="hps", bufs=2, space="PSUM") as hpsp, \
             tc.tile_pool(name="accps", bufs=1, space="PSUM") as accp, \
             tc.tile_pool(name="lps", bufs=1, space="PSUM") as lpsp:

            w1t_sb = constp.tile([128, KCH * H], bf)
            nc.gpsimd.dma_start(w1t_sb[:], w1t[:, :])
            w2r_sb = constp.tile([128, NBLK * H], bf)
            nc.gpsimd.dma_start(w2r_sb[:], w2r[:, :])
            ones_sb = constp.tile([128, 1], bf)
            nc.gpsimd.dma_start(ones_sb[:], ones[:, :])

            warm_sb = constp.tile([128, 256], bf)
            nc.gpsimd.memset(warm_sb[:], 0.0)
            warm_ps = hpsp.tile([128, NBLK * H], f32, tag="h_ps")
            for _ in range(28):
                nc.tensor.matmul(
                    warm_ps[:, 0:H], warm_sb[:, 0:128], warm_sb[:, 0:H],
                    start=True, stop=True, skip_group_check=True,
                )

            for s in range(SPC):
                acc0 = accp.tile([1, 512], f32, tag="acc0")
                acc1 = accp.tile([1, 512], f32, tag="acc1")
                l4_ps = lpsp.tile([NBLK, 1], f32, tag="l4")

                def emit_wsum(e_sb, xn_sb, t):
                    for b in range(NBLK):
                        first = (t == 0 and b == 0)
                        last = (t == TILES - 1 and b == NBLK - 1)
                        e_col = e_sb[:, b:b + 1]
                        nc.tensor.matmul(
                            acc0[:], e_col, xn_sb[:, b * D: b * D + 512],
                            start=first, stop=last, skip_group_check=True,
                        )
                        nc.tensor.matmul(
                            acc1[:], e_col, xn_sb[:, b * D + 512: (b + 1) * D],
                            start=first, stop=last, skip_group_check=True,
                        )
                    nc.tensor.matmul(
                        l4_ps[:], e_sb[:, 0:NBLK], ones_sb[:],
                        start=(t == 0), stop=(t == TILES - 1),
                        skip_group_check=True,
                    )

                prev = None
                for t in range(TILES):
                    n0 = t * NT
                    xt_sb = xtp.tile([128, KCH * NT], bf)
                    nc.sync.dma_start(xt_sb[:], xt[s, t])
                    xn_sb = xnp.tile([128, NBLK * D], bf)
                    nc.sync.dma_start(xn_sb[:], xn[s, t])
                    h_ps = hpsp.tile([128, NBLK * H], f32)
                    for b in range(NBLK):
                        for k in range(KCH):
                            nc.tensor.matmul(
                                h_ps[:, b * H:(b + 1) * H],
                                xt_sb[:, k * NT + b * 128: k * NT + b * 128 + 128],
                                w1t_sb[:, k * H:(k + 1) * H],
                                start=(k == 0), stop=(k == KCH - 1),
                            )
                    t_sb = tp.tile([128, NBLK * H], bf)
                    nc.scalar.activation(t_sb[:], h_ps[:], AF.Tanh)
                    tmp_sb = tmpp.tile([128, NBLK * H], bf)
                    nc.vector.tensor_mul(tmp_sb[:], t_sb[:], w2r_sb[:])
                    s_sb = sp.tile([128, NBLK], f32)
                    nc.vector.reduce_sum(
                        s_sb[:],
                        tmp_sb[:].rearrange("p (b j) -> p b j", b=NBLK),
                        axis=mybir.AxisListType.X,
                    )
                    e_sb = ep.tile([128, NBLK], bf)
                    nc.scalar.activation(e_sb[:], s_sb[:], AF.Exp)
                    if prev is not None:
                        emit_wsum(*prev)
                    prev = (e_sb, xn_sb, t)
                emit_wsum(*prev)

                l4_sb = outsbp.tile([NBLK, 1], f32, tag="l4sb")
                nc.scalar.activation(l4_sb[:], l4_ps[:], AF.Copy)
                nc.gpsimd.dma_start(outl[s, :, :], l4_sb[:])
                o_sb = outsbp.tile([1, D], f32, tag="osb")
                nc.scalar.activation(o_sb[:, 0:512], acc0[:], AF.Copy)
                nc.scalar.activation(o_sb[:, 512:D], acc1[:], AF.Copy)
                nc.gpsimd.dma_start(outp[s:s + 1, :], o_sb[:])

    nc.compile()
    return nc


def _get_nc():
    if "nc" not in _NC_CACHE:
        _NC_CACHE["nc"] = _build_nc()
    return _NC_CACHE["nc"]


def _prep_inputs(tiles_embeddings, W1, W2):
    X_bf = tiles_embeddings.astype(BF16)
    # xt[b, t, p, k, j] = X[b, t*NT + j, k*128 + p]
    xt_sw = np.ascontiguousarray(
        X_bf.reshape(B, TILES, NT, KCH, 128).transpose(0, 1, 4, 3, 2)
    ).reshape(B, TILES, 128, KCH * NT)
    # xn[b, t, p, bb, j] = X[b, t*NT + bb*128 + p, j]
    xn_sw = np.ascontiguousarray(
        X_bf.reshape(B, TILES, NBLK, 128, D).transpose(0, 1, 3, 2, 4)
    ).reshape(B, TILES, 128, NBLK * D)
    # w1t[p, k, h] = W1[h, k*128 + p]
    w1t = np.ascontiguousarray(
        W1.astype(BF16).reshape(H, KCH, 128).transpose(2, 1, 0)
    ).reshape(128, KCH * H)
    w2r = np.tile(W2.astype(BF16), (128, NBLK))
    ones = np.ones((128, 1), BF16)
    return [
        {
            "xt": xt_sw[c * SPC:(c + 1) * SPC],
            "xn": xn_sw[c * SPC:(c + 1) * SPC],
            "w1t": w1t,
            "w2r": w2r,
            "ones": ones,
        }
        for c in range(NCORES)
    ]


def _run(tiles_embeddings, W1, W2, **spmd_kwargs):
    nc = _get_nc()
    in_maps = _prep_inputs(tiles_embeddings, W1, W2)
    res = run_bass_kernel_spmd(nc, in_maps, core_ids=list(range(NCORES)), **spmd_kwargs)
    acc = np.concatenate([r["out"] for r in res.results], axis=0)      # [B, D]
    l4 = np.concatenate([r["outl"] for r in res.results], axis=0)      # [B, 4, 1]
    out = acc / l4.sum(axis=(1, 2), keepdims=False)[:, None]
    return out.astype(np.float32, copy=False), res


def kernel(tiles_embeddings, W1, W2):
    out, _ = _run(
        np.asarray(tiles_embeddings), np.asarray(W1), np.asarray(W2)
    )
    return out



# revision 6
# speedup vs baseline: 1.0432x; 1.0432x over previous
"""AttentionMIL pooling kernel for 8 Trainium2 NeuronCores.

Math (per slide b): h = tanh(X @ W1^T); s = h @ w2; a = softmax(s);
out = a^T @ X, with X [N=8192, D=1024], W1 [H=256, D], w2 [H].

Strategy (single-copy, ~33.6 MB/core HBM vs 67 MB for the two-layout
baseline):
  - Data-parallel over the slide dim: 16 slides / 8 cores = 2 per core.
  - Host ships ONLY the transposed bf16 layout xt (d on partitions, rows on
    free). Scores per 1024-row tile: ht = W1t-stationary @ xt on PE
    (32 matmuls of F=512 into a 4-bank fp32 PSUM tile), one tanh per
    h-chunk (ACT).
  - The w2 contraction is a PE matmul whose stationary is w2 REPLICATED
    across 128 columns: out [128, 1024] has every partition equal to the
    score row, i.e. the scores arrive already broadcast across partitions
    for the same cost (matmul cost depends only on F). exp on ACT fuses the
    PSUM->SBUF copy and emits accum_out = sum_j exp(s_j) (softmax
    denominator; no max-subtraction needed: |s| <= ||w2||_1 ~ 13 cannot
    overflow fp32).
  - Weighted sum WITHOUT a second X layout: per d-chunk k,
    r[p, k] += sum_j xt[p, k*1024+j] * e[j]. Chunks 0-4 via fused
    scalar_tensor_tensor on DVE (mul + free-dim reduce + [P,1] accum in one
    1x op). Chunks 5-7: one 2x tensor_tensor premultiply on DVE, then three
    activation-accumulate reduces on ACT. This splits the reduction so DVE
    (~117us), ACT (~116us) and PE (~124us) all stay near the ridge.
    Per-(chunk, tile) partials land in r_all slots; one tiny reduce per
    slide folds them; host divides by the denominator.
  - The score tail for tile t is emitted after the ht matmuls of tile t+1
    so the PE never stalls waiting on ACT; tanh is split per h-chunk so the
    single-buffer ht PSUM tile frees early for tile t+1.
"""

import sys

sys.path.insert(0, "/opt/trn_rl_repo")

import numpy as np
import ml_dtypes

import concourse.bacc as bacc
import concourse.tile as tile
from concourse import mybir
from concourse.bass_utils import run_bass_kernel_spmd

BF16 = ml_dtypes.bfloat16
B, N, D, H = 16, 8192, 1024, 256
NCORES = 8
SPC = B // NCORES          # slides per core
NT = 1024                  # rows of N per tile
TILES = N // NT
KCH = D // 128             # d-chunks (contraction blocks)
HCH = H // 128             # h-chunks
FH = NT // 512             # 512-wide F slices per tile (PSUM bank limit)
NDVE = 5                   # weighted-sum chunks via DVE STT; rest premul+ACT

_NC_CACHE = {}


def _build_nc():
    bf = mybir.dt.bfloat16
    f32 = mybir.dt.float32
    AF = mybir.ActivationFunctionType
    ALU = mybir.AluOpType

    nc = bacc.Bacc("TRN2", num_devices=NCORES)
    # Host-swizzled: each per-tile DMA reads one contiguous 2 MiB region
    # into a [128, 8192] SBUF tile (128 descriptors x 16 KiB).
    #   xt[s, t, p, k*NT + j] = X[s, t*NT + j, k*128 + p]
    xt = nc.declare_dram_parameter("xt", [SPC, TILES, 128, KCH * NT], bf, isOutput=False)
    #   w1t[p, k*H + h] = W1[h, k*128 + p]
    w1t = nc.declare_dram_parameter("w1t", [128, KCH * H], bf, isOutput=False)
    #   w2rep[p, hc*128 + m] = W2[0, hc*128 + p]  (column-replicated blocks)
    w2rep = nc.declare_dram_parameter("w2rep", [128, HCH * 128], bf, isOutput=False)
    # out[s, p, k] = sum_n e_n X[n, k*128+p]  for k<8;  out[s, p, 8] = l
    outp = nc.declare_dram_parameter("out", [SPC, 128, KCH + 1], f32, isOutput=True)

    with tile.TileContext(nc) as tc:
        with tc.tile_pool(name="const", bufs=1) as constp, \
             tc.tile_pool(name="xt", bufs=4) as xtp, \
             tc.tile_pool(name="th", bufs=2) as thp, \
             tc.tile_pool(name="ebc", bufs=3) as ebcp, \
             tc.tile_pool(name="tmp3", bufs=2) as tmp3p, \
             tc.tile_pool(name="scr", bufs=1) as scrp, \
             tc.tile_pool(name="acc", bufs=1) as accp, \
             tc.tile_pool(name="osb", bufs=2) as osbp, \
             tc.tile_pool(name="htps", bufs=1, space="PSUM") as htpsp, \
             tc.tile_pool(name="sbc", bufs=2, space="PSUM") as sbcp:

            w1t_sb = constp.tile([128, KCH * H], bf)
            nc.gpsimd.dma_start(w1t_sb[:], w1t[:, :])
            w2rep_sb = constp.tile([128, HCH * 128], bf)
            nc.gpsimd.dma_start(w2rep_sb[:], w2rep[:, :])

            # scratch for op outputs whose only consumed result is accum_out
            scr_dve = scrp.tile([128, NT], bf)
            scr_act = scrp.tile([128, NT], bf)

            for s in range(SPC):
                # per-(chunk, tile) weighted-sum partials + per-tile exp sums
                r_all = accp.tile([128, KCH * TILES], f32, tag=f"racc{s}")
                l_all = accp.tile([128, TILES], f32, tag=f"lacc{s}")

                def tail(u, th_sb, xt_sb):
                    s_bc = sbcp.tile([128, NT], f32)
                    for f in range(FH):
                        for hc in range(HCH):
                            nc.tensor.matmul(
                                s_bc[:, f * 512:(f + 1) * 512],
                                w2rep_sb[:, hc * 128:(hc + 1) * 128],
                                th_sb[:, hc * NT + f * 512: hc * NT + f * 512 + 512],
                                start=(hc == 0), stop=(hc == HCH - 1),
                            )
                    e_bc = ebcp.tile([128, NT], bf)
                    nc.scalar.activation(
                        e_bc[:], s_bc[:], AF.Exp,
                        accum_out=l_all[:, u:u + 1],
                    )
                    for k in range(NDVE):
                        nc.vector.scalar_tensor_tensor(
                            out=scr_dve[:],
                            in0=xt_sb[:, k * NT:(k + 1) * NT],
                            scalar=1.0,
                            in1=e_bc[:],
                            op0=ALU.mult,
                            op1=ALU.mult,
                            accum_out=r_all[:, k * TILES + u:k * TILES + u + 1],
                        )
                    # chunks NDVE..KCH: 2x premultiplies on DVE, ACT reduces
                    tmp3 = tmp3p.tile([128, (KCH - NDVE) * NT], bf)
                    for i in range(KCH - NDVE):
                        nc.vector.tensor_mul(
                            tmp3[:, i * NT:(i + 1) * NT],
                            xt_sb[:, (NDVE + i) * NT:(NDVE + i + 1) * NT],
                            e_bc[:],
                        )
                    for i in range(KCH - NDVE):
                        k = NDVE + i
                        nc.scalar.activation(
                            scr_act[:], tmp3[:, i * NT:(i + 1) * NT],
                            AF.Copy,
                            accum_out=r_all[:, k * TILES + u:k * TILES + u + 1],
                        )

                prev = None
                for t in range(TILES):
                    xt_sb = xtp.tile([128, KCH * NT], bf)
                    nc.sync.dma_start(xt_sb[:], xt[s, t])
                    ht_ps = htpsp.tile([128, HCH * NT], f32)
                    for hc in range(HCH):
                        for f in range(FH):
                            for k in range(KCH):
                                nc.tensor.matmul(
                                    ht_ps[:, hc * NT + f * 512: hc * NT + f * 512 + 512],
                                    w1t_sb[:, k * H + hc * 128: k * H + hc * 128 + 128],
                                    xt_sb[:, k * NT + f * 512: k * NT + f * 512 + 512],
                                    start=(k == 0), stop=(k == KCH - 1),
                                )
                    th_sb = thp.tile([128, HCH * NT], bf)
                    for hc in range(HCH):
                        nc.scalar.activation(
                            th_sb[:, hc * NT:(hc + 1) * NT],
                            ht_ps[:, hc * NT:(hc + 1) * NT],
                            AF.Tanh,
                        )
                    if prev is not None:
                        tail(*prev)
                    prev = (t, th_sb, xt_sb)
                tail(*prev)

                o_sb = osbp.tile([128, KCH + 1], f32)
                nc.vector.reduce_sum(
                    o_sb[:, 0:KCH],
                    r_all[:].rearrange("p (k t) -> p k t", k=KCH),
                    axis=mybir.AxisListType.X,
                )
                nc.vector.reduce_sum(
                    o_sb[:, KCH:KCH + 1],
                    l_all[:].rearrange("p (o t) -> p o t", o=1),
                    axis=mybir.AxisListType.X,
                )
                nc.scalar.dma_start(outp[s], o_sb[:])

    nc.compile()
    return nc


def _get_nc():
    if "nc" not in _NC_CACHE:
        _NC_CACHE["nc"] = _build_nc()
    return _NC_CACHE["nc"]


def _prep_inputs(tiles_embeddings, W1, W2):
    X_bf = tiles_embeddings.astype(BF16)
    # xt[b, t, p, k, j] = X[b, t*NT + j, k*128 + p]
    xt_sw = np.ascontiguousarray(
        X_bf.reshape(B, TILES, NT, KCH, 128).transpose(0, 1, 4, 3, 2)
    ).reshape(B, TILES, 128, KCH * NT)
    # w1t[p, k, h] = W1[h, k*128 + p]
    w1t = np.ascontiguousarray(
        W1.astype(BF16).reshape(H, KCH, 128).transpose(2, 1, 0)
    ).reshape(128, KCH * H)
    # w2rep[p, hc*128 + m] = W2[0, hc*128 + p]
    w2c = W2.astype(BF16).reshape(HCH, 128)
    w2rep = np.ascontiguousarray(
        np.repeat(w2c[:, :, None], 128, axis=2).transpose(1, 0, 2)
    ).reshape(128, HCH * 128)
    return [
        {
            "xt": xt_sw[c * SPC:(c + 1) * SPC],
            "w1t": w1t,
            "w2rep": w2rep,
        }
        for c in range(NCORES)
    ]


def _run(tiles_embeddings, W1, W2, **spmd_kwargs):
    nc = _get_nc()
    in_maps = _prep_inputs(tiles_embeddings, W1, W2)
    res = run_bass_kernel_spmd(nc, in_maps, core_ids=list(range(NCORES)), **spmd_kwargs)
    raw = np.concatenate([r["out"] for r in res.results], axis=0)  # [B, 128, 9]
    acc = raw[:, :, 0:KCH].transpose(0, 2, 1).reshape(B, D)        # d = k*128 + p
    l = raw[:, 0, KCH]                                             # [B]
    out = acc / l[:, None]
    return out.astype(np.float32, copy=False), res


def kernel(tiles_embeddings, W1, W2):
    out, _ = _run(
        np.asarray(tiles_embeddings), np.asarray(W1), np.asarray(W2)
    )
    return out


# revision 9
# speedup vs baseline: 1.1024x; 1.0567x over previous
"""AttentionMIL pooling kernel for 8 Trainium2 NeuronCores.

Math (per slide b): h = tanh(X @ W1^T); s = h @ w2; a = softmax(s);
out = a^T @ X, with X [N=8192, D=1024], W1 [H=256, D], w2 [H].

Strategy (single-copy, ~33.6 MB/core HBM vs 67 MB for the two-layout
baseline):
  - Data-parallel over the slide dim: 16 slides / 8 cores = 2 per core.
  - Host ships ONLY the transposed bf16 layout xt (d on partitions, rows on
    free). Scores per 1024-row tile: ht = W1t-stationary @ xt on PE
    (32 matmuls of F=512 into a 4-bank fp32 PSUM tile), one tanh per
    h-chunk (ACT).
  - The w2 contraction is a PE matmul whose stationary is w2 REPLICATED
    across 128 columns: out [128, 1024] has every partition equal to the
    score row, i.e. the scores arrive already broadcast across partitions
    for the same cost (matmul cost depends only on F). exp on ACT fuses the
    PSUM->SBUF copy and emits accum_out = sum_j exp(s_j) (softmax
    denominator; no max-subtraction needed: |s| <= ||w2||_1 ~ 13 cannot
    overflow fp32).
  - Weighted sum WITHOUT a second X layout: per d-chunk k,
    r[p, k] += sum_j xt[p, k*1024+j] * e[j]. Chunks 0-4 via fused
    scalar_tensor_tensor on DVE (mul + free-dim reduce + [P,1] accum in one
    1x op). Chunks 5-7: one 2x tensor_tensor premultiply on DVE, then three
    activation-accumulate reduces on ACT. This splits the reduction so DVE
    (~117us), ACT (~116us) and PE (~124us) all stay near the ridge.
    Per-(chunk, tile) partials land in r_all slots; one tiny reduce per
    slide folds them; host divides by the denominator.
  - The score tail for tile t is emitted after the ht matmuls of tile t+1
    so the PE never stalls waiting on ACT; tanh is split per h-chunk so the
    single-buffer ht PSUM tile frees early for tile t+1.
"""

import sys

sys.path.insert(0, "/opt/trn_rl_repo")

import numpy as np
import ml_dtypes

import concourse.bacc as bacc
import concourse.tile as tile
from concourse import mybir
from concourse.bass_utils import run_bass_kernel_spmd

BF16 = ml_dtypes.bfloat16
B, N, D, H = 16, 8192, 1024, 256
NCORES = 8
SPC = B // NCORES          # slides per core
NT = 1024                  # rows of N per tile
TILES = N // NT
KCH = D // 128             # d-chunks (contraction blocks)
HCH = H // 128             # h-chunks
FH = NT // 512             # 512-wide F slices per tile (PSUM bank limit)
NDVE = 5                   # weighted-sum chunks via DVE STT; rest premul+ACT

_NC_CACHE = {}


def _build_nc():
    bf = mybir.dt.bfloat16
    f32 = mybir.dt.float32
    AF = mybir.ActivationFunctionType
    ALU = mybir.AluOpType

    nc = bacc.Bacc("TRN2", num_devices=NCORES)
    # Host-swizzled: each per-tile DMA reads one contiguous 2 MiB region
    # into a [128, 8192] SBUF tile (128 descriptors x 16 KiB).
    #   xt[s, t, p, k*NT + j] = X[s, t*NT + j, k*128 + p]
    xt = nc.declare_dram_parameter("xt", [SPC, TILES, 128, KCH * NT], bf, isOutput=False)
    #   w1t[p, k*H + h] = W1[h, k*128 + p]
    w1t = nc.declare_dram_parameter("w1t", [128, KCH * H], bf, isOutput=False)
    #   w2rep[p, hc*128 + m] = W2[0, hc*128 + p]  (column-replicated blocks)
    w2rep = nc.declare_dram_parameter("w2rep", [128, HCH * 128], bf, isOutput=False)
    # out[s, p, k] = sum_n e_n X[n, k*128+p]  for k<8;  out[s, p, 8] = l
    outp = nc.declare_dram_parameter("out", [SPC, 128, KCH + 1], f32, isOutput=True)

    with tile.TileContext(nc) as tc:
        with tc.tile_pool(name="const", bufs=1) as constp, \
             tc.tile_pool(name="xt", bufs=3) as xtp, \
             tc.tile_pool(name="th", bufs=3) as thp, \
             tc.tile_pool(name="ebc", bufs=3) as ebcp, \
             tc.tile_pool(name="tmp3", bufs=2) as tmp3p, \
             tc.tile_pool(name="scr", bufs=1) as scrp, \
             tc.tile_pool(name="acc", bufs=1) as accp, \
             tc.tile_pool(name="osb", bufs=2) as osbp, \
             tc.tile_pool(name="htps", bufs=2, space="PSUM") as htpsp, \
             tc.tile_pool(name="warmps", bufs=1, space="PSUM") as warmpsp, \
             tc.tile_pool(name="sbc", bufs=2, space="PSUM") as sbcp:

            w1t_sb = constp.tile([128, KCH * H], bf)
            nc.gpsimd.dma_start(w1t_sb[:], w1t[:, :])
            w2rep_sb = constp.tile([128, HCH * 128], bf)
            nc.gpsimd.dma_start(w2rep_sb[:], w2rep[:, :])

            # p-state ramp: keep the PE busy ~3us so it reaches full clock
            # before the real matmuls (overlaps the first xt DMA).
            warm_sb = constp.tile([128, 256], bf)
            nc.gpsimd.memset(warm_sb[:], 0.0)
            warm_ps = warmpsp.tile([128, 512], f32)
            for _ in range(28):
                nc.tensor.matmul(
                    warm_ps[:, 0:256], warm_sb[:, 0:128], warm_sb[:, 0:256],
                    start=True, stop=True, skip_group_check=True,
                )

            # scratch for op outputs whose only consumed result is accum_out
            scr_dve = scrp.tile([128, NT], bf)
            scr_act = scrp.tile([128, NT], bf)

            for s in range(SPC):
                # per-(chunk, tile) weighted-sum partials + per-tile exp sums
                r_all = accp.tile([128, KCH * TILES], f32, tag=f"racc{s}")
                l_all = accp.tile([128, FH * TILES], f32, tag=f"lacc{s}")

                # score tail for one 512-row half: w2-matmul + exp
                def score_tail(u, f, th_sb, e_bc):
                    s_bc = sbcp.tile([128, 512], f32)
                    for hc in range(HCH):
                        nc.tensor.matmul(
                            s_bc[:],
                            w2rep_sb[:, hc * 128:(hc + 1) * 128],
                            th_sb[:, hc * 512:(hc + 1) * 512],
                            start=(hc == 0), stop=(hc == HCH - 1),
                        )
                    nc.scalar.activation(
                        e_bc[:, f * 512:(f + 1) * 512], s_bc[:], AF.Exp,
                        accum_out=l_all[:, 2 * u + f:2 * u + f + 1],
                    )

                # weighted-sum ops for one 1024-row group
                def wsum(u, xt_sb, e_bc):
                    for k in range(NDVE):
                        nc.vector.scalar_tensor_tensor(
                            out=scr_dve[:],
                            in0=xt_sb[:, k * NT:(k + 1) * NT],
                            scalar=1.0,
                            in1=e_bc[:],
                            op0=ALU.mult,
                            op1=ALU.mult,
                            accum_out=r_all[:, k * TILES + u:k * TILES + u + 1],
                        )
                    tmp3 = tmp3p.tile([128, (KCH - NDVE) * NT], bf)
                    for i in range(KCH - NDVE):
                        nc.vector.tensor_mul(
                            tmp3[:, i * NT:(i + 1) * NT],
                            xt_sb[:, (NDVE + i) * NT:(NDVE + i + 1) * NT],
                            e_bc[:],
                        )
                    for i in range(KCH - NDVE):
                        k = NDVE + i
                        nc.scalar.activation(
                            scr_act[:], tmp3[:, i * NT:(i + 1) * NT],
                            AF.Copy,
                            accum_out=r_all[:, k * TILES + u:k * TILES + u + 1],
                        )

                pend_score = None   # (u, f, th_sb, e_bc) half awaiting tail
                pend_wsum = None    # (u, xt_sb, e_bc) group awaiting wsum
                e_bc = None
                for t in range(TILES):
                    xt_sb = xtp.tile([128, KCH * NT], bf)
                    nc.sync.dma_start(xt_sb[:], xt[s, t])
                    e_bc = ebcp.tile([128, NT], bf, name="e_bc")
                    for f in range(FH):
                        ht_ps = htpsp.tile([128, HCH * 512], f32)
                        for hc in range(HCH):
                            for k in range(KCH):
                                nc.tensor.matmul(
                                    ht_ps[:, hc * 512:(hc + 1) * 512],
                                    w1t_sb[:, k * H + hc * 128: k * H + hc * 128 + 128],
                                    xt_sb[:, k * NT + f * 512: k * NT + f * 512 + 512],
                                    start=(k == 0), stop=(k == KCH - 1),
                                )
                        th_sb = thp.tile([128, HCH * 512], bf, name="th_sb")
                        for hc in range(HCH):
                            nc.scalar.activation(
                                th_sb[:, hc * 512:(hc + 1) * 512],
                                ht_ps[:, hc * 512:(hc + 1) * 512],
                                AF.Tanh,
                            )
                        if pend_score is not None:
                            score_tail(*pend_score)
                        pend_score = (t, f, th_sb, e_bc)
                        if pend_wsum is not None:
                            wsum(*pend_wsum)
                            pend_wsum = None
                    pend_wsum = (t, xt_sb, e_bc)
                score_tail(*pend_score)
                wsum(*pend_wsum)

                o_sb = osbp.tile([128, KCH + 1], f32)
                nc.vector.reduce_sum(
                    o_sb[:, 0:KCH],
                    r_all[:].rearrange("p (k t) -> p k t", k=KCH),
                    axis=mybir.AxisListType.X,
                )
                nc.vector.reduce_sum(
                    o_sb[:, KCH:KCH + 1],
                    l_all[:].rearrange("p (o t) -> p o t", o=1),
                    axis=mybir.AxisListType.X,
                )
                nc.scalar.dma_start(outp[s], o_sb[:])

    nc.compile()
    return nc


def _get_nc():
    if "nc" not in _NC_CACHE:
        _NC_CACHE["nc"] = _build_nc()
    return _NC_CACHE["nc"]


def _prep_inputs(tiles_embeddings, W1, W2):
    X_bf = tiles_embeddings.astype(BF16)
    # xt[b, t, p, k, j] = X[b, t*NT + j, k*128 + p]
    xt_sw = np.ascontiguousarray(
        X_bf.reshape(B, TILES, NT, KCH, 128).transpose(0, 1, 4, 3, 2)
    ).reshape(B, TILES, 128, KCH * NT)
    # w1t[p, k, h] = W1[h, k*128 + p]
    w1t = np.ascontiguousarray(
        W1.astype(BF16).reshape(H, KCH, 128).transpose(2, 1, 0)
    ).reshape(128, KCH * H)
    # w2rep[p, hc*128 + m] = W2[0, hc*128 + p]
    w2c = W2.astype(BF16).reshape(HCH, 128)
    w2rep = np.ascontiguousarray(
        np.repeat(w2c[:, :, None], 128, axis=2).transpose(1, 0, 2)
    ).reshape(128, HCH * 128)
    return [
        {
            "xt": xt_sw[c * SPC:(c + 1) * SPC],
            "w1t": w1t,
            "w2rep": w2rep,
        }
        for c in range(NCORES)
    ]


def _run(tiles_embeddings, W1, W2, **spmd_kwargs):
    nc = _get_nc()
    in_maps = _prep_inputs(tiles_embeddings, W1, W2)
    res = run_bass_kernel_spmd(nc, in_maps, core_ids=list(range(NCORES)), **spmd_kwargs)
    raw = np.concatenate([r["out"] for r in res.results], axis=0)  # [B, 128, 9]
    acc = raw[:, :, 0:KCH].transpose(0, 2, 1).reshape(B, D)        # d = k*128 + p
    l = raw[:, 0, KCH]                                             # [B]
    out = acc / l[:, None]
    return out.astype(np.float32, copy=False), res


def kernel(tiles_embeddings, W1, W2):
    out, _ = _run(
        np.asarray(tiles_embeddings), np.asarray(W1), np.asarray(W2)
    )
    return out


# revision 10
# speedup vs baseline: 1.1940x; 1.0831x over previous
"""AttentionMIL pooling kernel for 8 Trainium2 NeuronCores.

Math (per slide b): h = tanh(X @ W1^T); s = h @ w2; a = softmax(s);
out = a^T @ X, with X [N=8192, D=1024], W1 [H=256, D], w2 [H].

Strategy (single-copy, ~33.6 MB/core HBM vs 67 MB for the two-layout
baseline):
  - Data-parallel over the slide dim: 16 slides / 8 cores = 2 per core.
  - Host ships ONLY the transposed bf16 layout xt (d on partitions, rows on
    free). Scores per 1024-row tile: ht = W1t-stationary @ xt on PE
    (32 matmuls of F=512 into a 4-bank fp32 PSUM tile), one tanh per
    h-chunk (ACT).
  - The w2 contraction is a PE matmul whose stationary is w2 REPLICATED
    across 128 columns: out [128, 1024] has every partition equal to the
    score row, i.e. the scores arrive already broadcast across partitions
    for the same cost (matmul cost depends only on F). exp on ACT fuses the
    PSUM->SBUF copy and emits accum_out = sum_j exp(s_j) (softmax
    denominator; no max-subtraction needed: |s| <= ||w2||_1 ~ 13 cannot
    overflow fp32).
  - Weighted sum WITHOUT a second X layout: per d-chunk k,
    r[p, k] += sum_j xt[p, k*1024+j] * e[j]. Chunks 0-4 via fused
    scalar_tensor_tensor on DVE (mul + free-dim reduce + [P,1] accum in one
    1x op). Chunks 5-7: one 2x tensor_tensor premultiply on DVE, then three
    activation-accumulate reduces on ACT. This splits the reduction so DVE
    (~117us), ACT (~116us) and PE (~124us) all stay near the ridge.
    Per-(chunk, tile) partials land in r_all slots; one tiny reduce per
    slide folds them; host divides by the denominator.
  - The score tail for tile t is emitted after the ht matmuls of tile t+1
    so the PE never stalls waiting on ACT; tanh is split per h-chunk so the
    single-buffer ht PSUM tile frees early for tile t+1.
"""

import sys

sys.path.insert(0, "/opt/trn_rl_repo")

import numpy as np
import ml_dtypes

import concourse.bacc as bacc
import concourse.tile as tile
from concourse import mybir
from concourse.bass_utils import run_bass_kernel_spmd

BF16 = ml_dtypes.bfloat16
B, N, D, H = 16, 8192, 1024, 256
NCORES = 8
SPC = B // NCORES          # slides per core
NT = 2048                  # rows of N per wsum super-group
TILES = N // NT
KCH = D // 128             # d-chunks (contraction blocks)
HCH = H // 128             # h-chunks
FH = NT // 512             # 512-wide F slices per tile (PSUM bank limit)
NDVE = 5                   # weighted-sum chunks via DVE STT; rest premul+ACT

_NC_CACHE = {}


def _build_nc():
    bf = mybir.dt.bfloat16
    f32 = mybir.dt.float32
    AF = mybir.ActivationFunctionType
    ALU = mybir.AluOpType

    nc = bacc.Bacc("TRN2", num_devices=NCORES)
    # Host-swizzled: each per-tile DMA reads one contiguous 2 MiB region
    # into a [128, 8192] SBUF tile (128 descriptors x 16 KiB).
    #   xt[s, t, p, k*NT + j] = X[s, t*NT + j, k*128 + p]
    xt = nc.declare_dram_parameter("xt", [SPC, TILES, 128, KCH * NT], bf, isOutput=False)
    #   w1t[p, k*H + h] = W1[h, k*128 + p]
    w1t = nc.declare_dram_parameter("w1t", [128, KCH * H], bf, isOutput=False)
    #   w2rep[p, hc*128 + m] = W2[0, hc*128 + p]  (column-replicated blocks)
    w2rep = nc.declare_dram_parameter("w2rep", [128, HCH * 128], bf, isOutput=False)
    # out[s, p, k] = sum_n e_n X[n, k*128+p]  for k<8;  out[s, p, 8] = l
    outp = nc.declare_dram_parameter("out", [SPC, 128, KCH + 1], f32, isOutput=True)

    with tile.TileContext(nc) as tc:
        with tc.tile_pool(name="const", bufs=1) as constp, \
             tc.tile_pool(name="xt", bufs=3) as xtp, \
             tc.tile_pool(name="th", bufs=3) as thp, \
             tc.tile_pool(name="ebc", bufs=3) as ebcp, \
             tc.tile_pool(name="tmp3", bufs=2) as tmp3p, \
             tc.tile_pool(name="scr", bufs=1) as scrp, \
             tc.tile_pool(name="acc", bufs=1) as accp, \
             tc.tile_pool(name="osb", bufs=2) as osbp, \
             tc.tile_pool(name="htps", bufs=2, space="PSUM") as htpsp, \
             tc.tile_pool(name="warmps", bufs=1, space="PSUM") as warmpsp, \
             tc.tile_pool(name="sbc", bufs=2, space="PSUM") as sbcp:

            w1t_sb = constp.tile([128, KCH * H], bf)
            nc.gpsimd.dma_start(w1t_sb[:], w1t[:, :])
            w2rep_sb = constp.tile([128, HCH * 128], bf)
            nc.gpsimd.dma_start(w2rep_sb[:], w2rep[:, :])

            # p-state ramp: keep the PE busy ~3us so it reaches full clock
            # before the real matmuls (overlaps the first xt DMA).
            warm_sb = constp.tile([128, 256], bf)
            nc.gpsimd.memset(warm_sb[:], 0.0)
            warm_ps = warmpsp.tile([128, 512], f32)
            for _ in range(14):
                nc.tensor.matmul(
                    warm_ps[:, 0:256], warm_sb[:, 0:128], warm_sb[:, 0:256],
                    start=True, stop=True, skip_group_check=True,
                )

            # scratch for op outputs whose only consumed result is accum_out
            scr_dve = scrp.tile([128, NT], bf)
            scr_act = scrp.tile([128, NT], bf)

            for s in range(SPC):
                # per-(chunk, tile) weighted-sum partials + per-tile exp sums
                r_all = accp.tile([128, KCH * TILES], f32, tag=f"racc{s}")
                l_all = accp.tile([128, FH * TILES], f32, tag=f"lacc{s}")

                # score tail for one 512-row half: w2-matmul + exp
                def score_tail(u, f, th_sb, e_bc):
                    s_bc = sbcp.tile([128, 512], f32)
                    for hc in range(HCH):
                        nc.tensor.matmul(
                            s_bc[:],
                            w2rep_sb[:, hc * 128:(hc + 1) * 128],
                            th_sb[:, hc * 512:(hc + 1) * 512],
                            start=(hc == 0), stop=(hc == HCH - 1),
                        )
                    nc.scalar.activation(
                        e_bc[:, f * 512:(f + 1) * 512], s_bc[:], AF.Exp,
                        accum_out=l_all[:, FH * u + f:FH * u + f + 1],
                    )

                # weighted-sum ops for one 1024-row group
                def wsum(u, xt_sb, e_bc):
                    for k in range(NDVE):
                        nc.vector.scalar_tensor_tensor(
                            out=scr_dve[:],
                            in0=xt_sb[:, k * NT:(k + 1) * NT],
                            scalar=1.0,
                            in1=e_bc[:],
                            op0=ALU.mult,
                            op1=ALU.mult,
                            accum_out=r_all[:, k * TILES + u:k * TILES + u + 1],
                        )
                    tmp3 = tmp3p.tile([128, (KCH - NDVE) * NT], bf)
                    for i in range(KCH - NDVE):
                        nc.vector.tensor_mul(
                            tmp3[:, i * NT:(i + 1) * NT],
                            xt_sb[:, (NDVE + i) * NT:(NDVE + i + 1) * NT],
                            e_bc[:],
                        )
                    for i in range(KCH - NDVE):
                        k = NDVE + i
                        nc.scalar.activation(
                            scr_act[:], tmp3[:, i * NT:(i + 1) * NT],
                            AF.Copy,
                            accum_out=r_all[:, k * TILES + u:k * TILES + u + 1],
                        )

                pend_score = None   # (u, f, th_sb, e_bc) half awaiting tail
                pend_wsum = None    # (u, xt_sb, e_bc) group awaiting wsum
                e_bc = None
                for t in range(TILES):
                    xt_sb = xtp.tile([128, KCH * NT], bf)
                    nc.sync.dma_start(xt_sb[:], xt[s, t])
                    e_bc = ebcp.tile([128, NT], bf, name="e_bc")
                    for f in range(FH):
                        ht_ps = htpsp.tile([128, HCH * 512], f32)
                        for hc in range(HCH):
                            for k in range(KCH):
                                nc.tensor.matmul(
                                    ht_ps[:, hc * 512:(hc + 1) * 512],
                                    w1t_sb[:, k * H + hc * 128: k * H + hc * 128 + 128],
                                    xt_sb[:, k * NT + f * 512: k * NT + f * 512 + 512],
                                    start=(k == 0), stop=(k == KCH - 1),
                                )
                        th_sb = thp.tile([128, HCH * 512], bf, name="th_sb")
                        nc.scalar.activation(th_sb[:], ht_ps[:], AF.Tanh)
                        if pend_score is not None:
                            score_tail(*pend_score)
                        pend_score = (t, f, th_sb, e_bc)
                        if pend_wsum is not None:
                            wsum(*pend_wsum)
                            pend_wsum = None
                    pend_wsum = (t, xt_sb, e_bc)
                score_tail(*pend_score)
                wsum(*pend_wsum)

                o_sb = osbp.tile([128, KCH + 1], f32)
                nc.vector.reduce_sum(
                    o_sb[:, 0:KCH],
                    r_all[:].rearrange("p (k t) -> p k t", k=KCH),
                    axis=mybir.AxisListType.X,
                )
                nc.vector.reduce_sum(
                    o_sb[:, KCH:KCH + 1],
                    l_all[:].rearrange("p (o t) -> p o t", o=1),
                    axis=mybir.AxisListType.X,
                )
                nc.scalar.dma_start(outp[s], o_sb[:])

    nc.compile()
    return nc


def _get_nc():
    if "nc" not in _NC_CACHE:
        _NC_CACHE["nc"] = _build_nc()
    return _NC_CACHE["nc"]


def _prep_inputs(tiles_embeddings, W1, W2):
    X_bf = tiles_embeddings.astype(BF16)
    # xt[b, t, p, k, j] = X[b, t*NT + j, k*128 + p]
    xt_sw = np.ascontiguousarray(
        X_bf.reshape(B, TILES, NT, KCH, 128).transpose(0, 1, 4, 3, 2)
    ).reshape(B, TILES, 128, KCH * NT)
    # w1t[p, k, h] = W1[h, k*128 + p]
    w1t = np.ascontiguousarray(
        W1.astype(BF16).reshape(H, KCH, 128).transpose(2, 1, 0)
    ).reshape(128, KCH * H)
    # w2rep[p, hc*128 + m] = W2[0, hc*128 + p]
    w2c = W2.astype(BF16).reshape(HCH, 128)
    w2rep = np.ascontiguousarray(
        np.repeat(w2c[:, :, None], 128, axis=2).transpose(1, 0, 2)
    ).reshape(128, HCH * 128)
    return [
        {
            "xt": xt_sw[c * SPC:(c + 1) * SPC],
            "w1t": w1t,
            "w2rep": w2rep,
        }
        for c in range(NCORES)
    ]


def _run(tiles_embeddings, W1, W2, **spmd_kwargs):
    nc = _get_nc()
    in_maps = _prep_inputs(tiles_embeddings, W1, W2)
    res = run_bass_kernel_spmd(nc, in_maps, core_ids=list(range(NCORES)), **spmd_kwargs)
    raw = np.concatenate([r["out"] for r in res.results], axis=0)  # [B, 128, 9]
    acc = raw[:, :, 0:KCH].transpose(0, 2, 1).reshape(B, D)        # d = k*128 + p
    l = raw[:, 0, KCH]                                             # [B]
    out = acc / l[:, None]
    return out.astype(np.float32, copy=False), res


def kernel(tiles_embeddings, W1, W2):
    out, _ = _run(
        np.asarray(tiles_embeddings), np.asarray(W1), np.asarray(W2)
    )
    return out


# revision 11
# speedup vs baseline: 1.2923x; 1.0823x over previous
"""AttentionMIL pooling kernel for 8 Trainium2 NeuronCores.

Math (per slide b): h = tanh(X @ W1^T); s = h @ w2; a = softmax(s);
out = a^T @ X, with X [N=8192, D=1024], W1 [H=256, D], w2 [H].

Strategy (single-copy, ~33.6 MB/core HBM vs 67 MB for the two-layout
baseline):
  - Data-parallel over the slide dim: 16 slides / 8 cores = 2 per core.
  - Host ships ONLY the transposed bf16 layout xt (d on partitions, rows on
    free). Scores per 1024-row tile: ht = W1t-stationary @ xt on PE
    (32 matmuls of F=512 into a 4-bank fp32 PSUM tile), one tanh per
    h-chunk (ACT).
  - The w2 contraction is a PE matmul whose stationary is w2 REPLICATED
    across 128 columns: out [128, 1024] has every partition equal to the
    score row, i.e. the scores arrive already broadcast across partitions
    for the same cost (matmul cost depends only on F). exp on ACT fuses the
    PSUM->SBUF copy and emits accum_out = sum_j exp(s_j) (softmax
    denominator; no max-subtraction needed: |s| <= ||w2||_1 ~ 13 cannot
    overflow fp32).
  - Weighted sum WITHOUT a second X layout: per d-chunk k,
    r[p, k] += sum_j xt[p, k*1024+j] * e[j]. Chunks 0-4 via fused
    scalar_tensor_tensor on DVE (mul + free-dim reduce + [P,1] accum in one
    1x op). Chunks 5-7: one 2x tensor_tensor premultiply on DVE, then three
    activation-accumulate reduces on ACT. This splits the reduction so DVE
    (~117us), ACT (~116us) and PE (~124us) all stay near the ridge.
    Per-(chunk, tile) partials land in r_all slots; one tiny reduce per
    slide folds them; host divides by the denominator.
  - The score tail for tile t is emitted after the ht matmuls of tile t+1
    so the PE never stalls waiting on ACT; tanh is split per h-chunk so the
    single-buffer ht PSUM tile frees early for tile t+1.
"""

import sys

sys.path.insert(0, "/opt/trn_rl_repo")

import numpy as np
import ml_dtypes

import concourse.bacc as bacc
import concourse.tile as tile
from concourse import mybir
from concourse.bass_utils import run_bass_kernel_spmd

BF16 = ml_dtypes.bfloat16
B, N, D, H = 16, 8192, 1024, 256
NCORES = 8
SPC = B // NCORES          # slides per core
NT = 2048                  # rows of N per wsum super-group
TILES = N // NT
KCH = D // 128             # d-chunks (contraction blocks)
HCH = H // 128             # h-chunks
FH = NT // 512             # 512-wide F slices per tile (PSUM bank limit)
NDVE = 5                   # weighted-sum chunks via DVE STT; rest premul+ACT
SLOTS = TILES + 1          # partial-sum slots (last group uses two)

_NC_CACHE = {}


def _build_nc():
    bf = mybir.dt.bfloat16
    f32 = mybir.dt.float32
    AF = mybir.ActivationFunctionType
    ALU = mybir.AluOpType

    nc = bacc.Bacc("TRN2", num_devices=NCORES)
    # Host-swizzled: each per-tile DMA reads one contiguous 2 MiB region
    # into a [128, 8192] SBUF tile (128 descriptors x 16 KiB).
    #   xt[s, t, p, k*NT + j] = X[s, t*NT + j, k*128 + p]
    xt = nc.declare_dram_parameter("xt", [SPC, TILES, 128, KCH * NT], bf, isOutput=False)
    #   w1t[p, k*H + h] = W1[h, k*128 + p]
    w1t = nc.declare_dram_parameter("w1t", [128, KCH * H], bf, isOutput=False)
    #   w2rep[p, hc*128 + m] = W2[0, hc*128 + p]  (column-replicated blocks)
    w2rep = nc.declare_dram_parameter("w2rep", [128, HCH * 128], bf, isOutput=False)
    # out[s, p, k] = sum_n e_n X[n, k*128+p]  for k<8;  out[s, p, 8] = l
    outp = nc.declare_dram_parameter("out", [SPC, 128, KCH + 1], f32, isOutput=True)

    with tile.TileContext(nc) as tc:
        with tc.tile_pool(name="const", bufs=1) as constp, \
             tc.tile_pool(name="xt", bufs=3) as xtp, \
             tc.tile_pool(name="th", bufs=3) as thp, \
             tc.tile_pool(name="ebc", bufs=3) as ebcp, \
             tc.tile_pool(name="tmp3", bufs=2) as tmp3p, \
             tc.tile_pool(name="scr", bufs=1) as scrp, \
             tc.tile_pool(name="acc", bufs=1) as accp, \
             tc.tile_pool(name="osb", bufs=2) as osbp, \
             tc.tile_pool(name="htps", bufs=2, space="PSUM") as htpsp, \
             tc.tile_pool(name="warmps", bufs=1, space="PSUM") as warmpsp, \
             tc.tile_pool(name="sbc", bufs=2, space="PSUM") as sbcp:

            w1t_sb = constp.tile([128, KCH * H], bf)
            nc.gpsimd.dma_start(w1t_sb[:], w1t[:, :])
            w2rep_sb = constp.tile([128, HCH * 128], bf)
            nc.gpsimd.dma_start(w2rep_sb[:], w2rep[:, :])

            # p-state ramp: keep the PE busy ~3us so it reaches full clock
            # before the real matmuls (overlaps the first xt DMA).
            warm_sb = constp.tile([128, 256], bf)
            nc.gpsimd.memset(warm_sb[:], 0.0)
            warm_ps = warmpsp.tile([128, 512], f32)
            for _ in range(14):
                nc.tensor.matmul(
                    warm_ps[:, 0:256], warm_sb[:, 0:128], warm_sb[:, 0:256],
                    start=True, stop=True, skip_group_check=True,
                )

            # scratch for op outputs whose only consumed result is accum_out
            scr_dve = scrp.tile([128, NT], bf)
            scr_act = scrp.tile([128, NT], bf)

            for s in range(SPC):
                # per-(chunk, tile) weighted-sum partials + per-tile exp sums
                r_all = accp.tile([128, KCH * SLOTS], f32, tag=f"racc{s}")
                l_all = accp.tile([128, FH * TILES], f32, tag=f"lacc{s}")

                # score tail for one 512-row half: w2-matmul + exp
                def score_tail(u, f, th_sb, e_bc):
                    s_bc = sbcp.tile([128, 512], f32)
                    for hc in range(HCH):
                        nc.tensor.matmul(
                            s_bc[:],
                            w2rep_sb[:, hc * 128:(hc + 1) * 128],
                            th_sb[:, hc * 512:(hc + 1) * 512],
                            start=(hc == 0), stop=(hc == HCH - 1),
                        )
                    nc.scalar.activation(
                        e_bc[:, f * 512:(f + 1) * 512], s_bc[:], AF.Exp,
                        accum_out=l_all[:, FH * u + f:FH * u + f + 1],
                    )

                # weighted-sum ops for rows [j0, j1) of a super-group,
                # accumulating into partial-sum slot `slot` (0..SLOTS-1)
                def wsum(slot, xt_sb, e_bc, j0, j1):
                    w = j1 - j0
                    for k in range(NDVE):
                        nc.vector.scalar_tensor_tensor(
                            out=scr_dve[:, 0:w],
                            in0=xt_sb[:, k * NT + j0:k * NT + j1],
                            scalar=1.0,
                            in1=e_bc[:, j0:j1],
                            op0=ALU.mult,
                            op1=ALU.mult,
                            accum_out=r_all[:, k * SLOTS + slot:k * SLOTS + slot + 1],
                        )
                    tmp3 = tmp3p.tile([128, (KCH - NDVE) * NT], bf, name="tmp3")
                    for i in range(KCH - NDVE):
                        nc.vector.tensor_mul(
                            tmp3[:, i * NT + j0:i * NT + j1],
                            xt_sb[:, (NDVE + i) * NT + j0:(NDVE + i) * NT + j1],
                            e_bc[:, j0:j1],
                        )
                    for i in range(KCH - NDVE):
                        k = NDVE + i
                        nc.scalar.activation(
                            scr_act[:, 0:w], tmp3[:, i * NT + j0:i * NT + j1],
                            AF.Copy,
                            accum_out=r_all[:, k * SLOTS + slot:k * SLOTS + slot + 1],
                        )

                pend_score = None   # (u, f, th_sb, e_bc) half awaiting tail
                pend_wsum = None    # (u, xt_sb, e_bc) group awaiting wsum
                e_bc = None
                last = TILES - 1
                for t in range(TILES):
                    xt_sb = xtp.tile([128, KCH * NT], bf)
                    for q in range(4):
                        qw = KCH * NT // 4
                        nc.sync.dma_start(
                            xt_sb[:, q * qw:(q + 1) * qw],
                            xt[s, t, :, q * qw:(q + 1) * qw],
                        )
                    e_bc = ebcp.tile([128, NT], bf, name="e_bc")
                    for f in range(FH):
                        ht_ps = htpsp.tile([128, HCH * 512], f32)
                        for hc in range(HCH):
                            for k in range(KCH):
                                nc.tensor.matmul(
                                    ht_ps[:, hc * 512:(hc + 1) * 512],
                                    w1t_sb[:, k * H + hc * 128: k * H + hc * 128 + 128],
                                    xt_sb[:, k * NT + f * 512: k * NT + f * 512 + 512],
                                    start=(k == 0), stop=(k == KCH - 1),
                                )
                        th_sb = thp.tile([128, HCH * 512], bf, name="th_sb")
                        nc.scalar.activation(th_sb[:], ht_ps[:], AF.Tanh)
                        if pend_score is not None:
                            score_tail(*pend_score)
                        pend_score = (t, f, th_sb, e_bc)
                        if pend_wsum is not None:
                            wsum(pend_wsum[0], pend_wsum[1], pend_wsum[2], 0, NT)
                            pend_wsum = None
                        if t == last and f == FH - 1:
                            # drain shrink: first half of the final group can
                            # start as soon as its first two exps are done
                            wsum(TILES - 1, xt_sb, e_bc, 0, NT // 2)
                    if t < last:
                        pend_wsum = (t, xt_sb, e_bc)
                score_tail(*pend_score)
                wsum(TILES, xt_sb, e_bc, NT // 2, NT)

                o_sb = osbp.tile([128, KCH + 1], f32)
                nc.vector.reduce_sum(
                    o_sb[:, 0:KCH],
                    r_all[:].rearrange("p (k t) -> p k t", k=KCH),
                    axis=mybir.AxisListType.X,
                )
                nc.vector.reduce_sum(
                    o_sb[:, KCH:KCH + 1],
                    l_all[:].rearrange("p (o t) -> p o t", o=1),
                    axis=mybir.AxisListType.X,
                )
                nc.scalar.dma_start(outp[s], o_sb[:])

    nc.compile()
    return nc


def _get_nc():
    if "nc" not in _NC_CACHE:
        _NC_CACHE["nc"] = _build_nc()
    return _NC_CACHE["nc"]


def _prep_inputs(tiles_embeddings, W1, W2):
    X_bf = tiles_embeddings.astype(BF16)
    # xt[b, t, p, k, j] = X[b, t*NT + j, k*128 + p]
    xt_sw = np.ascontiguousarray(
        X_bf.reshape(B, TILES, NT, KCH, 128).transpose(0, 1, 4, 3, 2)
    ).reshape(B, TILES, 128, KCH * NT)
    # w1t[p, k, h] = W1[h, k*128 + p]
    w1t = np.ascontiguousarray(
        W1.astype(BF16).reshape(H, KCH, 128).transpose(2, 1, 0)
    ).reshape(128, KCH * H)
    # w2rep[p, hc*128 + m] = W2[0, hc*128 + p]
    w2c = W2.astype(BF16).reshape(HCH, 128)
    w2rep = np.ascontiguousarray(
        np.repeat(w2c[:, :, None], 128, axis=2).transpose(1, 0, 2)
    ).reshape(128, HCH * 128)
    return [
        {
            "xt": xt_sw[c * SPC:(c + 1) * SPC],
            "w1t": w1t,
            "w2rep": w2rep,
        }
        for c in range(NCORES)
    ]


def _run(tiles_embeddings, W1, W2, **spmd_kwargs):
    nc = _get_nc()
    in_maps = _prep_inputs(tiles_embeddings, W1, W2)
    res = run_bass_kernel_spmd(nc, in_maps, core_ids=list(range(NCORES)), **spmd_kwargs)
    raw = np.concatenate([r["out"] for r in res.results], axis=0)  # [B, 128, 9]
    acc = raw[:, :, 0:KCH].transpose(0, 2, 1).reshape(B, D)        # d = k*128 + p
    l = raw[:, 0, KCH]                                             # [B]
    out = acc / l[:, None]
    return out.astype(np.float32, copy=False), res


def kernel(tiles_embeddings, W1, W2):
    out, _ = _run(
        np.asarray(tiles_embeddings), np.asarray(W1), np.asarray(W2)
    )
    return out
